# revision 50
# baseline (speedup 1.0000x reference)
"""Trainium2 Bass kernel for DeepConvGraphEncoderDownstream.

Model (per reference):
  4-layer GCN (shared dense 24x24 graph operator) applied per (batch, timestep)
  frame -> node-mean -> per sliding window (W=32, stride 2, 113 windows):
  BiLSTM(H=256) -> concat(h_fwd[-1], h_bwd[0]) @ Wfc + bfc.

Key restructurings vs the reference:
  * gcn_norm folded into one dense Ahat[24,24] on host; node-mix done as
    matmul with kron(Ahat^T, I5) over blk=(n*5+g5) packing.
  * GCN runs ONCE over all 256 timesteps (reference recomputes ~14x).
  * Alternating mix-first / transform-first layers: ONE XBAR DMA
    transpose per GCN layer (5 per chunk vs 9 before); layer 4's mix
    uses the activations as the matmul stationary operand, flipping
    B->A layout on the PE itself (no transpose; per-partition bias+relu
    lands on the A-layout output).
  * L2-L4 processed in half-chunks (26 gb) for finer pipelining and
    half-size tiles (SBUF).
  * LSTM gate biases folded into the precomputed input transform U.
  * backward LSTM: only hb[:, 0] is used => exactly ONE step.
  * forward LSTM: 113 windows batched into a 904-row recurrence, split
    into two independent 452-row groups alternating per step so one
    group's elementwise tail hides under the other's matmuls.

Sharding: data-parallel over batch, 8 batches/core on 8 cores.

Layouts (per core, per chunk = one local batch = 256 timesteps padded to
260 = 52 gb-blocks * 5):
  A-layout [c_part, free=(gb, blk:128)], blk = n*5+g5 (120:128 pad),
           timestep t = 5*gb + g5.
  B-layout [blk partitions (120 real), free=(gb, c)]
"""

import os
import sys
import numpy as np

try:
    import concourse.bass as bass
except ImportError:
    sys.path.insert(0, "/opt/trn_rl_repo")
    import concourse.bass as bass

import concourse.bacc as bacc
import concourse.tile as tile
from concourse import mybir
from concourse import bass_utils

F16 = mybir.dt.float16
F32 = mybir.dt.float32
AF = mybir.ActivationFunctionType
ALU = mybir.AluOpType

B, T, N, FIN = 64, 256, 24, 6
H, EMB = 256, 128
WIN = 32
NW = (T - WIN) // 2 + 1               # 113
NCORES = 8
BL = B // NCORES                      # 8
G5 = 5
GBLK = 52                             # 52*5 = 260 padded t-slots
HGB = GBLK // 2                       # 26 gb per half-chunk
TP = GBLK * G5                        # 260
NCH = BL
ROWS = BL * NW                        # 904
HROWS = ROWS // 2                     # 452
NB = N * G5                           # 120 real blk rows
C0 = 32                               # ch-pad of input (6 -> 32)
AFREE = GBLK * 128                    # 6656
HFREE = HGB * 128                     # 3328 free per half-chunk
FTOT = BL * TP                        # 2080 F columns

_CACHE = {}


def _kernel_body(tc, io):
    nc = tc.nc
    from contextlib import ExitStack
    ctx = ExitStack()

    cons = ctx.enter_context(tc.tile_pool(name="cons", bufs=1))
    fpool = ctx.enter_context(tc.tile_pool(name="fpool", bufs=1))

    def load_const(name, shape, dt=F16):
        t = cons.tile(shape, dt, name=name)
        nc.sync.dma_start(t[:], io[name][:])
        return t

    mixA = load_const("mixA", [128, 128])
    mixZ = load_const("mixZ", [128, 128])
    w1 = load_const("w1", [128, 64])
    w2 = load_const("w2", [64, 128])
    w3 = load_const("w3", [128, 256])
    b1 = load_const("b1", [64, 1], F32)
    b3 = load_const("b3", [128, 2], F32)
    b4 = load_const("b4", [128, 2], F32)
    ident = load_const("ident", [128, 128])
    w4k = []
    for kt in range(2):
        t = cons.tile([128, 256], F16, name=f"w4k{kt}")
        nc.sync.dma_start(t[:], io["w4"][kt * 128:(kt + 1) * 128, :])
        w4k.append(t)

    def load_ktiles(name):
        ts = []
        for kt in range(2):
            t = cons.tile([128, 1024], F16, name=f"{name}{kt}")
            nc.sync.dma_start(t[:], io[name][kt * 128:(kt + 1) * 128, :])
            ts.append(t)
        return ts

    lxf = load_ktiles("lxf")
    lhf = load_ktiles("lhf")
    lxb = load_ktiles("lxb")
    bgf = load_const("bgf", [128, 8], F32)
    bgb = load_const("bgb", [128, 8], F32)
    wfct = []
    for qt in range(4):
        t = cons.tile([128, 128], F16, name=f"wfct{qt}")
        nc.sync.dma_start(t[:], io["wfc"][qt * 128:(qt + 1) * 128, :])
        wfct.append(t)
    bfc = load_const("bfc", [128, 1], F32)

    F0 = fpool.tile([128, FTOT], F16, name="F0")
    F1 = fpool.tile([128, FTOT], F16, name="F1")
    Fts = [F0, F1]

    # ================= Phase 1: GCN =================
    # per chunk:
    #  L1 (mix-first, full chunk): mix@C0 (ws) -> T(B->A) -> tf C0->64
    #     (+b1, relu) into two half tiles
    #  per half-chunk:
    #  L2 (tf-first): tf 64->128 raw -> T(A->B) -> +b2row -> mix(+b2) ->
    #     relu
    #  L3 (mix-first): mix raw -> T(B->A) -> tf 128->256 (+b3, relu)
    #  L4 (tf-first): tf 256->256 raw -> T(A->B) x2 -> acts-stat mix
    #     (B->A on PE) -> +b4+relu -> node-sum (DVE) -> F
    TCHH = [(i * 512, 512) for i in range(6)] + [(3072, 256)]   # 3328
    QPC = [(0, 6), (6, 13), (13, 19), (19, 26)]   # transpose gb pieces

    with tc.tile_pool(name="gA", bufs=2) as gA, \
         tc.tile_pool(name="gA3", bufs=3) as gA3, \
         tc.tile_pool(name="gB", bufs=2) as gB, \
         tc.tile_pool(name="gB3", bufs=3) as gB3, \
         tc.tile_pool(name="gSm", bufs=2) as gSm, \
         tc.tile_pool(name="g1", bufs=1) as g1, \
         tc.tile_pool(name="pT", bufs=4, space="PSUM") as pT, \
         tc.tile_pool(name="pM", bufs=4, space="PSUM") as pM:

        # L1 packs 4 consecutive gb (a "G group", 4*32 ch-cols = 128) per
        # stationary load; (G, g4) lexicographic order == plain gb order.
        NG4 = GBLK // 4                  # 13 G groups per chunk
        for k in range(NCH):
            x0 = gSm.tile([128, GBLK * C0], F16, tag="x0", name="x0")
            nc.sync.dma_start(x0[:], io["x0B"][k])

            # --- L1: acts-stat mix (B->A on PE; 4 gb per stationary) ->
            #     tf C0->64 in four concurrent 32-row PE strips (+b1, relu)
            y1X = gSm.tile([128, NG4 * 128], F16, tag="y1X", name="y1X")
            for q in range(4):           # 13 G groups in bursts of 4
                ps = pM.tile([128, 512], F32, tag="mps", name="mps1")
                ng = min(4, NG4 - q * 4)
                for j in range(ng):
                    G = q * 4 + j
                    nc.tensor.matmul(
                        ps[:, j * 128:(j + 1) * 128],
                        x0[:, G * 128:(G + 1) * 128],
                        mixZ[:], start=True, stop=True)
                nc.vector.tensor_copy(
                    y1X[:, q * 512:q * 512 + ng * 128],
                    ps[:, 0:ng * 128])
            # tf: contract c (32) sits in partition strip g4*32; W1 is
            # replicated per strip in w1 [128, 64].
            x1F = g1.tile([64, AFREE], F16, tag="x1A", name="x1F")
            x1v = x1F[:].rearrange("p (G g4 blk) -> p G g4 blk",
                                   g4=4, blk=128)
            for g4 in range(4):
                for fc in range(4):      # 13 G groups in chunks of 4
                    G0 = fc * 4
                    nG = min(4, NG4 - G0)
                    ps = pT.tile([128, 512], F32, tag="tps", name="tps1")
                    nc.tensor.matmul(
                        ps[0:64, 0:nG * 128], w1[g4 * 32:(g4 + 1) * 32, :],
                        y1X[g4 * 32:(g4 + 1) * 32,
                            G0 * 128:(G0 + nG) * 128],
                        start=True, stop=True,
                        tile_position=(g4 * 32, 0))
                    psv = ps[0:64, 0:nG * 128].rearrange(
                        "p (G blk) -> p G blk", blk=128)
                    # G group G0+i covers gb 4*(G0+i)+g4
                    nc.scalar.activation(x1v[:, G0:G0 + nG, g4, :], psv,
                                         AF.Relu, bias=b1[:, 0:1], scale=1.0)
            x1A = [x1F[:, 0:HFREE], x1F[:, HFREE:AFREE]]

            def raw_copy(i, dst, src):
                if i % 2 == 0:
                    nc.vector.tensor_copy(dst, src)
                else:
                    nc.scalar.copy(dst, src)

            def relu_copy(i, dst, src, bias):
                if i % 2 == 0:
                    nc.scalar.activation(dst, src, AF.Relu,
                                         bias=bias, scale=1.0)
                else:
                    nc.vector.tensor_scalar(dst, src, bias, 0.0,
                                            ALU.add, ALU.max)

            # --- L2: tf 64->128 raw -> A->B -> +b2row -> mix(+b2) -> relu
            h2A_l, h2B_l, x2B_l = [], [], []
            for h in range(2):
                h2A = gA3.tile([128, HFREE], F16, tag="h2A", name="h2A")
                for i, (f0, fw) in enumerate(TCHH):
                    ps = pT.tile([128, 512], F32, tag="tps", name="tps2")
                    nc.tensor.matmul(ps[:, 0:fw], w2[:],
                                     x1A[h][:, f0:f0 + fw],
                                     start=True, stop=True)
                    raw_copy(i + h, h2A[:, f0:f0 + fw], ps[:, 0:fw])
                h2A_l.append(h2A)
            for h in range(2):
                h2B = gB3.tile([128, HFREE], F16, tag="h2B", name="h2B")
                nc.sync.dma_start(h2B[NB:NB + 1, :],
                                  io["b2row"][:, 0:HFREE])
                h2Bv = h2B[0:NB].rearrange("p (gb c) -> p gb c", c=128)
                for q0, q1 in QPC:
                    nc.sync.dma_start(h2Bv[:, q0:q1, :],
                                      h2A_l[h][:, q0 * 128:q1 * 128],
                                      transpose=True)
                h2B_l.append(h2B)
            for h in range(2):
                x2B = gB3.tile([128, HFREE], F16, tag="x2B", name="x2B")
                for i, (f0, fw) in enumerate(TCHH):
                    ps = pM.tile([128, 512], F32, tag="mps", name="mps2")
                    nc.tensor.matmul(ps[:, 0:fw], mixA[0:NB + 1, :],
                                     h2B_l[h][0:NB + 1, f0:f0 + fw],
                                     start=True, stop=True)
                    if (i + h) % 2 == 0:
                        nc.vector.tensor_scalar_max(x2B[:, f0:f0 + fw],
                                                    ps[:, 0:fw], 0.0)
                    else:
                        nc.scalar.activation(x2B[:, f0:f0 + fw],
                                             ps[:, 0:fw], AF.Relu)
                x2B_l.append(x2B)

            # --- L3: mix raw -> B->A -> tf 128->256 (+b3, relu)
            y3B_l, y3A_l, x3A_l = [], [], []
            for h in range(2):
                y3B = gB.tile([128, HFREE], F16, tag="y3B", name="y3B")
                for i, (f0, fw) in enumerate(TCHH):
                    ps = pM.tile([128, 512], F32, tag="mps", name="mps3")
                    nc.tensor.matmul(ps[:, 0:fw], mixZ[:],
                                     x2B_l[h][:, f0:f0 + fw],
                                     start=True, stop=True)
                    raw_copy(i + h, y3B[:, f0:f0 + fw], ps[:, 0:fw])
                y3B_l.append(y3B)
            for h in range(2):
                y3A = gA3.tile([128, HFREE], F16, tag="y3A", name="y3A")
                y3Av = y3A[:].rearrange("c (gb p) -> c gb p", p=128)
                for q0, q1 in QPC:
                    nc.sync.dma_start(y3Av[:, q0:q1, :],
                                      y3B_l[h][:, q0 * 128:q1 * 128],
                                      transpose=True)
                y3A_l.append(y3A)
            # --- L3 tf + L4 per half (L4: tf raw -> T(A->B) x2 ->
            #     acts-stat mix (B->A on PE) -> +b4+relu -> tree sum)
            for h in range(2):
                ho = h * HGB
                x3A = []
                for mt in range(2):
                    xt = gA.tile([128, HFREE], F16, tag="x3A",
                                 name=f"x3A{mt}")
                    for i, (f0, fw) in enumerate(TCHH):
                        ps = pT.tile([128, 512], F32, tag="tps", name="tps3")
                        nc.tensor.matmul(ps[:, 0:fw],
                                         w3[:, mt * 128:(mt + 1) * 128],
                                         y3A_l[h][:, f0:f0 + fw],
                                         start=True, stop=True)
                        relu_copy(i + mt, xt[:, f0:f0 + fw], ps[:, 0:fw],
                                  b3[:, mt:mt + 1])
                    x3A.append(xt)
                x3A_l.append(x3A)
                h4B = []
                for mt in range(2):
                    h4A = gA.tile([128, HFREE], F16, tag="h4A",
                                  name=f"h4A{mt}")
                    for i, (f0, fw) in enumerate(TCHH):
                        ps = pT.tile([128, 512], F32, tag="tps", name="tps4")
                        for kt in range(2):
                            nc.tensor.matmul(
                                ps[:, 0:fw],
                                w4k[kt][:, mt * 128:(mt + 1) * 128],
                                x3A_l[h][kt][:, f0:f0 + fw],
                                start=(kt == 0), stop=(kt == 1))
                        raw_copy(i + mt, h4A[:, f0:f0 + fw], ps[:, 0:fw])
                    hb = gB.tile([128, HFREE], F16, tag="h4B",
                                 name=f"h4B{mt}")
                    hbv = hb[0:NB].rearrange("p (gb c) -> p gb c", c=128)
                    for q0, q1 in QPC:
                        nc.sync.dma_start(hbv[:, q0:q1, :],
                                          h4A[:, q0 * 128:q1 * 128],
                                          transpose=True)
                    h4B.append(hb)
                nsum = g1.tile([128, HGB * 90], F16, tag="nsum",
                               name="nsum")
                for cb in range(2):
                    xt = gSm.tile([128, HGB * NB], F16, tag="x4A",
                                  name=f"x4A{cb}")
                    for gq, g0 in enumerate(range(0, HGB, 4)):
                        ng = min(4, HGB - g0)
                        ps = pM.tile([128, 512], F32, tag="mps",
                                     name="aps")
                        for j in range(ng):
                            gb = g0 + j
                            nc.tensor.matmul(
                                ps[:, j * NB:(j + 1) * NB],
                                h4B[cb][0:NB, gb * 128:gb * 128 + 128],
                                mixA[0:NB, 0:NB], start=True, stop=True)
                        dv = xt[:, g0 * NB:(g0 + ng) * NB]
                        relu_copy(gq + cb, dv, ps[:, 0:ng * NB],
                                  b4[:, cb:cb + 1])
                    # tree node-sum into F: 24 nodes -> 12 -> 6 -> 3 -> 1
                    xv = xt[:].rearrange("p (gb blk) -> p gb blk", blk=NB)
                    t1 = nsum[:, 0:HGB * 60].rearrange(
                        "p (gb x) -> p gb x", x=60)
                    t2 = nsum[:, HGB * 60:HGB * 90].rearrange(
                        "p (gb x) -> p gb x", x=30)
                    nc.gpsimd.tensor_tensor(t1, xv[:, :, 0:60],
                                            xv[:, :, 60:120], ALU.add)
                    nc.gpsimd.tensor_tensor(t2, t1[:, :, 0:30],
                                            t1[:, :, 30:60], ALU.add)
                    t3 = t1[:, :, 0:15]
                    nc.gpsimd.tensor_tensor(t3, t2[:, :, 0:15],
                                            t2[:, :, 15:30], ALU.add)
                    dstv = Fts[cb][:, k * TP + ho * G5:
                                   k * TP + (ho + HGB) * G5].rearrange(
                        "p (gb g5) -> p gb g5", g5=G5)
                    nc.gpsimd.tensor_tensor(t2[:, :, 0:5], t3[:, :, 0:5],
                                            t3[:, :, 5:10], ALU.add)
                    nc.gpsimd.tensor_tensor(dstv, t2[:, :, 0:5],
                                            t3[:, :, 10:15], ALU.add)

    # ===== Phase 2: U = F @ (Wih_f/24)^T + (bih+bhh)  (bias folded) =====
    upool = ctx.enter_context(tc.tile_pool(name="upool", bufs=1))
    UCH = [(i * 512, 512) for i in range(4)] + [(2048, 32)]
    Umt = []
    with tc.tile_pool(name="ups", bufs=3, space="PSUM") as ps_u:
        for mt in range(8):
            u = upool.tile([128, FTOT], F16, name=f"U{mt}")
            for f0, fw in UCH:
                ps = ps_u.tile([128, 512], F32, tag="ups", name="ups")
                for kt in range(2):
                    nc.tensor.matmul(ps[:, 0:fw],
                                     lxf[kt][:, mt * 128:(mt + 1) * 128],
                                     Fts[kt][:, f0:f0 + fw],
                                     start=(kt == 0), stop=(kt == 1))
                dst = u[:, f0:f0 + fw]
                src = ps[:, 0:fw]
                if mt % 2:
                    nc.scalar.activation(dst, src, AF.Identity,
                                         bias=bgf[:, mt:mt + 1], scale=1.0)
                else:
                    nc.vector.tensor_scalar_add(dst, src, bgf[:, mt:mt + 1])
            Umt.append(u)

    # ================= Phase 3: forward LSTM (two row groups) ============
    lp = ctx.enter_context(tc.tile_pool(name="lstm", bufs=1))
    NG = 2
    GR = ROWS // NG                      # 452
    Hf = [lp.tile([128, 2 * GR], F16, name=f"Hf{g}") for g in range(NG)]
    Cf = [lp.tile([128, 2 * GR], F16, name=f"Cf{g}") for g in range(NG)]
    gates = [[lp.tile([128, 2 * GR], F16, name=f"g{g}_{p}")
              for p in range(4)] for g in range(NG)]
    tmp = [lp.tile([128, 2 * GR], F16, name=f"tmp{g}") for g in range(NG)]
    tcl = [lp.tile([128, 2 * GR], F16, name=f"tcl{g}") for g in range(NG)]
    for g in range(NG):
        nc.vector.memset(Hf[g][:], 0.0)
        nc.vector.memset(Cf[g][:], 0.0)

    with tc.tile_pool(name="lps", bufs=4, space="PSUM") as ps_l:
        for s in range(WIN):
            k0, par = s // 2, s % 2
            for g in range(NG):
                b0 = g * (BL // NG)
                for mp in range(4):
                    ps = ps_l.tile([128, 1024], F32, tag="lps", name="lps")
                    for j in range(2):
                        mt = 2 * mp + j
                        pslice = ps[:, j * 512:j * 512 + GR]
                        uv = Umt[mt][:].rearrange(
                            "p (b k two) -> p b k two", b=BL, two=2)
                        nc.tensor.matmul(
                            pslice, ident[:],
                            uv[:, b0:b0 + BL // NG, k0:k0 + NW, par],
                            start=True, stop=False)
                        for kt in range(2):
                            nc.tensor.matmul(
                                pslice, lhf[kt][:, mt * 128:(mt + 1) * 128],
                                Hf[g][:, kt * GR:(kt + 1) * GR],
                                start=False, stop=(kt == 1))
                    fn = AF.Sigmoid if mp < 3 else AF.Tanh
                    dstv = gates[g][mp][:].rearrange("p (j r) -> p j r", j=2)
                    psv = ps[:].rearrange("p (j x) -> p j x", j=2)[:, :, 0:GR]
                    nc.scalar.activation(dstv, psv, fn)
                gi, gf, go, tg = gates[g]
                nc.vector.tensor_tensor(tmp[g][:], gi[:], tg[:], ALU.mult)
                nc.vector.tensor_tensor(Cf[g][:], gf[:], Cf[g][:], ALU.mult)
                nc.vector.tensor_tensor(Cf[g][:], Cf[g][:], tmp[g][:],
                                        ALU.add)
                nc.scalar.activation(tcl[g][:], Cf[g][:], AF.Tanh)
                nc.vector.tensor_tensor(Hf[g][:], go[:], tcl[g][:], ALU.mult)

        # ===== Phase 4: backward LSTM single step (only hb[:,0] used) =====
        Hb = [lp.tile([128, 2 * GR], F16, name=f"Hb{g}") for g in range(NG)]
        kb = (WIN - 2) // 2
        for g in range(NG):
            b0 = g * (BL // NG)
            for mp in [0, 2, 3]:          # forget gate irrelevant (c0=0)
                ps = ps_l.tile([128, 1024], F32, tag="lps", name="lpsb")
                for j in range(2):
                    mt = 2 * mp + j
                    pslice = ps[:, j * 512:j * 512 + GR]
                    for kt in range(2):
                        fv = Fts[kt][:].rearrange(
                            "p (b k two) -> p b k two", b=BL, two=2)
                        nc.tensor.matmul(
                            pslice, lxb[kt][:, mt * 128:(mt + 1) * 128],
                            fv[:, b0:b0 + BL // NG, kb:kb + NW, 1],
                            start=(kt == 0), stop=(kt == 1))
                fn = AF.Sigmoid if mp < 3 else AF.Tanh
                for j in range(2):
                    mt = 2 * mp + j
                    nc.scalar.activation(
                        gates[g][mp][:, j * GR:(j + 1) * GR],
                        ps[:, j * 512:j * 512 + GR], fn,
                        bias=bgb[:, mt:mt + 1], scale=1.0)
            gi, gf, go, tg = gates[g]
            nc.vector.tensor_tensor(tmp[g][:], gi[:], tg[:], ALU.mult)
            nc.scalar.activation(tcl[g][:], tmp[g][:], AF.Tanh)
            nc.vector.tensor_tensor(Hb[g][:], go[:], tcl[g][:], ALU.mult)

        # ===== Phase 5: FC head =====
        for g in range(NG):
            ps = ps_l.tile([128, 1024], F32, tag="lps", name=f"lpsf{g}")
            rhs4 = [Hf[g][:, 0:GR], Hf[g][:, GR:2 * GR],
                    Hb[g][:, 0:GR], Hb[g][:, GR:2 * GR]]
            for qt in range(4):
                nc.tensor.matmul(ps[:, 0:GR], wfct[qt][:], rhs4[qt],
                                 start=(qt == 0), stop=(qt == 3))
            ob = lp.tile([EMB, GR], F32, name=f"ob{g}")
            nc.scalar.activation(ob[:], ps[:, 0:GR], AF.Identity,
                                 bias=bfc[:, 0:1], scale=1.0)
            nc.sync.dma_start(io["out_d"][:, g * GR:(g + 1) * GR], ob[:])

    ctx.close()


def _build_program():
    nc = bacc.Bacc("TRN2", target_bir_lowering=False, debug=False,
                   num_devices=NCORES)

    def din(name, shape, dt=F16):
        return nc.dram_tensor(name, shape, dt, kind="ExternalInput").ap()

    io = dict(
        x0B=din("x0B", [NCH, 128, GBLK * C0]),
        mixA=din("mixA", [128, 128]), mixZ=din("mixZ", [128, 128]),
        w1=din("w1", [128, 64]), w2=din("w2", [64, 128]),
        w3=din("w3", [128, 256]), w4=din("w4", [256, 256]),
        b1=din("b1", [64, 1], F32), b2row=din("b2row", [1, AFREE]),
        b3=din("b3", [128, 2], F32), b4=din("b4", [128, 2], F32),
        lxf=din("lxf", [256, 1024]), lhf=din("lhf", [256, 1024]),
        lxb=din("lxb", [256, 1024]),
        bgf=din("bgf", [128, 8], F32), bgb=din("bgb", [128, 8], F32),
        wfc=din("wfc", [512, 128]), bfc=din("bfc", [128, 1], F32),
        ident=din("ident", [128, 128]),
        out_d=nc.dram_tensor("out", [EMB, ROWS], F32,
                             kind="ExternalOutput").ap(),
    )
    with tile.TileContext(nc) as tc:
        _kernel_body(tc, io)
    nc.compile()
    return nc


def _host_prep(inputs):
    f16 = np.float16
    data = np.asarray(inputs["data"], np.float32)
    ei = np.asarray(inputs["edge_index"]).astype(np.int64)

    src = np.concatenate([ei[0], np.arange(N)])
    dst = np.concatenate([ei[1], np.arange(N)])
    deg = np.zeros(N, np.float32)
    np.add.at(deg, dst, 1.0)
    dinv = np.where(deg > 0, deg ** -0.5, 0.0).astype(np.float32)
    Ahat = np.zeros((N, N), np.float32)
    np.add.at(Ahat, (dst, src), dinv[src] * dinv[dst])
    mixZ = np.zeros((128, 128), np.float32)
    mixZ[0:NB, 0:NB] = np.kron(Ahat.T, np.eye(G5, dtype=np.float32))
    mixA = mixZ.copy()
    mixA[NB, 0:NB] = 1.0                 # bias row for L2's mix
    mixA = mixA.astype(f16)
    mixZ = mixZ.astype(f16)

    # x0B: [chunk b][blk = n*5+g5 (120:128 zero)][gb*C0 + c], t = 5*gb+g5
    d = data.reshape(NCORES, BL, T, N, FIN)
    x0B = np.zeros((NCORES, BL, 128, GBLK, C0), np.float32)
    dpad = np.zeros((NCORES, BL, TP, N, FIN), np.float32)
    dpad[:, :, :T] = d
    dv = dpad.reshape(NCORES, BL, GBLK, G5, N, FIN)
    dv = dv.transpose(0, 1, 4, 3, 2, 5).reshape(NCORES, BL, NB, GBLK, FIN)
    x0B[:, :, 0:NB, :, 0:FIN] = dv
    x0B = np.ascontiguousarray(
        x0B.reshape(NCORES, BL, 128, GBLK * C0)).astype(f16)

    perm = np.concatenate([np.arange(0, H), np.arange(H, 2 * H),
                           np.arange(3 * H, 4 * H), np.arange(2 * H, 3 * H)])

    def prep_dir(wih, whh, bih, bhh):
        wihp = np.asarray(wih, np.float32)[perm] / N
        whhp = np.asarray(whh, np.float32)[perm]
        bg = (np.asarray(bih, np.float32) + np.asarray(bhh, np.float32))[perm]
        return (np.ascontiguousarray(wihp.T).astype(f16),
                np.ascontiguousarray(whhp.T).astype(f16),
                np.ascontiguousarray(bg.reshape(8, 128).T).astype(np.float32))

    lxf, lhf, bgf = prep_dir(inputs["lstm_Wih_f"], inputs["lstm_Whh_f"],
                             inputs["lstm_bih_f"], inputs["lstm_bhh_f"])
    lxb, _lhb, bgb = prep_dir(inputs["lstm_Wih_b"], inputs["lstm_Whh_b"],
                              inputs["lstm_bih_b"], inputs["lstm_bhh_b"])

    b2 = np.asarray(inputs["b2"], np.float32)
    b2row = np.tile(b2, GBLK).reshape(1, AFREE).astype(f16)

    com = {
        "mixA": mixA, "mixZ": mixZ,
        "w1": np.tile(np.pad(np.asarray(inputs["W1"], np.float32),
                             ((0, C0 - FIN), (0, 0))), (4, 1)).astype(f16),
        "w2": np.asarray(inputs["W2"], np.float32).astype(f16),
        "w3": np.asarray(inputs["W3"], np.float32).astype(f16),
        "w4": np.asarray(inputs["W4"], np.float32).astype(f16),
        "b1": np.asarray(inputs["b1"], np.float32).reshape(64, 1),
        "b2row": b2row,
        "b3": np.ascontiguousarray(
            np.asarray(inputs["b3"], np.float32).reshape(2, 128).T),
        "b4": np.ascontiguousarray(
            np.asarray(inputs["b4"], np.float32).reshape(2, 128).T),
        "lxf": lxf, "lhf": lhf, "lxb": lxb, "bgf": bgf, "bgb": bgb,
        "wfc": np.asarray(inputs["Wfc"], np.float32).astype(f16),
        "bfc": np.asarray(inputs["bfc"], np.float32).reshape(128, 1),
        "ident": np.eye(128, dtype=f16),
    }
    return [dict(com, x0B=x0B[c]) for c in range(NCORES)]


TRACE = False          # set by test harness to capture an NTFF profile


def kernel(**inputs) -> np.ndarray:
    if "nc" not in _CACHE:
        _CACHE["nc"] = _build_program()
    nc = _CACHE["nc"]
    in_maps = _host_prep(inputs)
    res = bass_utils.run_bass_kernel_spmd(nc, in_maps,
                                          core_ids=list(range(NCORES)),
                                          trace=TRACE)
    _CACHE["last_res"] = res
    outs = []
    for c in range(NCORES):
        o = res.results[c]["out"]                       # [128, 904]
        outs.append(o.reshape(EMB, BL, NW).transpose(1, 2, 0))
    return np.concatenate(outs, 0).astype(np.float32)   # [64, 113, 128]


if __name__ == "__main__":
    import reference
    ins = {k: np.asarray(v) for k, v in reference.setup_inputs().items()}
    out = kernel(**ins)
    print("kernel out", out.shape, out.dtype, float(np.abs(out).max()))


# revision 55
# speedup vs baseline: 1.0361x; 1.0361x over previous
"""Trainium2 Bass kernel for DeepConvGraphEncoderDownstream.

Model (per reference):
  4-layer GCN (shared dense 24x24 graph operator) applied per (batch, timestep)
  frame -> node-mean -> per sliding window (W=32, stride 2, 113 windows):
  BiLSTM(H=256) -> concat(h_fwd[-1], h_bwd[0]) @ Wfc + bfc.

Key restructurings vs the reference:
  * gcn_norm folded into one dense Ahat[24,24] on host; node-mix done as
    matmul with kron(Ahat^T, I5) over blk=(n*5+g5) packing.
  * GCN runs ONCE over all 256 timesteps (reference recomputes ~14x).
  * Alternating mix-first / transform-first layers: ONE XBAR DMA
    transpose per GCN layer (5 per chunk vs 9 before); layer 4's mix
    uses the activations as the matmul stationary operand, flipping
    B->A layout on the PE itself (no transpose; per-partition bias+relu
    lands on the A-layout output).
  * L2-L4 processed in half-chunks (26 gb) for finer pipelining and
    half-size tiles (SBUF).
  * LSTM gate biases folded into the precomputed input transform U.
  * backward LSTM: only hb[:, 0] is used => exactly ONE step.
  * forward LSTM: 113 windows batched into a 904-row recurrence, split
    into two independent 452-row groups alternating per step so one
    group's elementwise tail hides under the other's matmuls.

Sharding: data-parallel over batch, 8 batches/core on 8 cores.

Layouts (per core, per chunk = one local batch = 256 timesteps padded to
260 = 52 gb-blocks * 5):
  A-layout [c_part, free=(gb, blk:128)], blk = n*5+g5 (120:128 pad),
           timestep t = 5*gb + g5.
  B-layout [blk partitions (120 real), free=(gb, c)]
"""

import os
import sys
import numpy as np

try:
    import concourse.bass as bass
except ImportError:
    sys.path.insert(0, "/opt/trn_rl_repo")
    import concourse.bass as bass

import concourse.bacc as bacc
import concourse.tile as tile
from concourse import mybir
from concourse import bass_utils

F16 = mybir.dt.float16
F32 = mybir.dt.float32
AF = mybir.ActivationFunctionType
ALU = mybir.AluOpType

B, T, N, FIN = 64, 256, 24, 6
H, EMB = 256, 128
WIN = 32
NW = (T - WIN) // 2 + 1               # 113
NCORES = 8
BL = B // NCORES                      # 8
G5 = 5
GBLK = 52                             # 52*5 = 260 padded t-slots
HGB = GBLK // 2                       # 26 gb per half-chunk
TP = GBLK * G5                        # 260
NCH = BL
ROWS = BL * NW                        # 904
HROWS = ROWS // 2                     # 452
NB = N * G5                           # 120 real blk rows
C0 = 32                               # ch-pad of input (6 -> 32)
AFREE = GBLK * 128                    # 6656
HFREE = HGB * 128                     # 3328 free per half-chunk
FTOT = BL * TP                        # 2080 F columns

_CACHE = {}


def _kernel_body(tc, io):
    nc = tc.nc
    from contextlib import ExitStack
    ctx = ExitStack()

    cons = ctx.enter_context(tc.tile_pool(name="cons", bufs=1))
    fpool = ctx.enter_context(tc.tile_pool(name="fpool", bufs=1))

    def load_const(name, shape, dt=F16):
        t = cons.tile(shape, dt, name=name)
        nc.sync.dma_start(t[:], io[name][:])
        return t

    mixA = load_const("mixA", [128, 128])
    mixZ = load_const("mixZ", [128, 128])
    w1 = load_const("w1", [128, 64])
    w2 = load_const("w2", [64, 128])
    w3 = load_const("w3", [128, 256])
    b1 = load_const("b1", [64, 1], F32)
    b3 = load_const("b3", [128, 2], F32)
    b4 = load_const("b4", [128, 2], F32)
    ident = load_const("ident", [128, 128])
    w4k = []
    for kt in range(2):
        t = cons.tile([128, 256], F16, name=f"w4k{kt}")
        nc.sync.dma_start(t[:], io["w4"][kt * 128:(kt + 1) * 128, :])
        w4k.append(t)

    def load_ktiles(name):
        ts = []
        for kt in range(2):
            t = cons.tile([128, 1024], F16, name=f"{name}{kt}")
            nc.sync.dma_start(t[:], io[name][kt * 128:(kt + 1) * 128, :])
            ts.append(t)
        return ts

    lxf = load_ktiles("lxf")
    lhf = load_ktiles("lhf")
    lxb = load_ktiles("lxb")
    bgf = load_const("bgf", [128, 8], F32)
    bgb = load_const("bgb", [128, 8], F32)
    wfct = []
    for qt in range(4):
        t = cons.tile([128, 128], F16, name=f"wfct{qt}")
        nc.sync.dma_start(t[:], io["wfc"][qt * 128:(qt + 1) * 128, :])
        wfct.append(t)
    bfc = load_const("bfc", [128, 1], F32)

    F0 = fpool.tile([128, FTOT], F16, name="F0")
    F1 = fpool.tile([128, FTOT], F16, name="F1")
    Fts = [F0, F1]

    # ================= Phase 1: GCN =================
    # per chunk:
    #  L1 (mix-first, full chunk): mix@C0 (ws) -> T(B->A) -> tf C0->64
    #     (+b1, relu) into two half tiles
    #  per half-chunk:
    #  L2 (tf-first): tf 64->128 raw -> T(A->B) -> +b2row -> mix(+b2) ->
    #     relu
    #  L3 (mix-first): mix raw -> T(B->A) -> tf 128->256 (+b3, relu)
    #  L4 (tf-first): tf 256->256 raw -> T(A->B) x2 -> acts-stat mix
    #     (B->A on PE) -> +b4+relu -> node-sum (DVE) -> F
    TCHH = [(i * 512, 512) for i in range(6)] + [(3072, 256)]   # 3328
    QPC = [(0, 6), (6, 13), (13, 19), (19, 26)]   # transpose gb pieces

    with tc.tile_pool(name="gA", bufs=2) as gA, \
         tc.tile_pool(name="gA3", bufs=3) as gA3, \
         tc.tile_pool(name="gB", bufs=2) as gB, \
         tc.tile_pool(name="gB3", bufs=3) as gB3, \
         tc.tile_pool(name="gB4", bufs=4) as gB4, \
         tc.tile_pool(name="gSm", bufs=2) as gSm, \
         tc.tile_pool(name="g1", bufs=1) as g1, \
         tc.tile_pool(name="pT", bufs=3, space="PSUM") as pT, \
         tc.tile_pool(name="pM", bufs=3, space="PSUM") as pM, \
         tc.tile_pool(name="pA", bufs=2, space="PSUM") as pA:

        # L1 packs 4 consecutive gb (a "G group", 4*32 ch-cols = 128) per
        # stationary load; (G, g4) lexicographic order == plain gb order.
        NG4 = GBLK // 4                  # 13 G groups per chunk
        for k in range(NCH):
            x0 = gSm.tile([128, GBLK * C0], F16, tag="x0", name="x0")
            nc.sync.dma_start(x0[:], io["x0B"][k])

            # --- L1: acts-stat mix (B->A on PE; 4 gb per stationary) ->
            #     tf C0->64 in four concurrent 32-row PE strips (+b1, relu)
            y1X = gSm.tile([128, NG4 * 128], F16, tag="y1X", name="y1X")
            for q in range(4):           # 13 G groups in bursts of 4
                ps = pM.tile([128, 512], F32, tag="mps", name="mps1")
                ng = min(4, NG4 - q * 4)
                for j in range(ng):
                    G = q * 4 + j
                    nc.tensor.matmul(
                        ps[:, j * 128:(j + 1) * 128],
                        x0[:, G * 128:(G + 1) * 128],
                        mixZ[:], start=True, stop=True)
                nc.vector.tensor_copy(
                    y1X[:, q * 512:q * 512 + ng * 128],
                    ps[:, 0:ng * 128])
            # tf: contract c (32) sits in partition strip g4*32; W1 is
            # replicated per strip in w1 [128, 64].
            x1F = g1.tile([64, AFREE], F16, tag="x1A", name="x1F")
            x1v = x1F[:].rearrange("p (G g4 blk) -> p G g4 blk",
                                   g4=4, blk=128)
            for g4 in range(4):
                for fc in range(4):      # 13 G groups in chunks of 4
                    G0 = fc * 4
                    nG = min(4, NG4 - G0)
                    ps = pT.tile([128, 512], F32, tag="tps", name="tps1")
                    nc.tensor.matmul(
                        ps[0:64, 0:nG * 128], w1[g4 * 32:(g4 + 1) * 32, :],
                        y1X[g4 * 32:(g4 + 1) * 32,
                            G0 * 128:(G0 + nG) * 128],
                        start=True, stop=True,
                        tile_position=(g4 * 32, 0))
                    psv = ps[0:64, 0:nG * 128].rearrange(
                        "p (G blk) -> p G blk", blk=128)
                    # G group G0+i covers gb 4*(G0+i)+g4
                    nc.scalar.activation(x1v[:, G0:G0 + nG, g4, :], psv,
                                         AF.Relu, bias=b1[:, 0:1], scale=1.0)
            x1A = [x1F[:, 0:HFREE], x1F[:, HFREE:AFREE]]

            def raw_copy(i, dst, src):
                if i % 2 == 0:
                    nc.vector.tensor_copy(dst, src)
                else:
                    nc.scalar.copy(dst, src)

            def relu_copy(i, dst, src, bias):
                if i % 2 == 0:
                    nc.scalar.activation(dst, src, AF.Relu,
                                         bias=bias, scale=1.0)
                else:
                    nc.vector.tensor_scalar(dst, src, bias, 0.0,
                                            ALU.add, ALU.max)

            # --- L2: tf 64->128 raw -> A->B -> +b2row -> mix(+b2) -> relu
            h2A_l, h2B_l, x2B_l = [], [], []
            for h in range(2):
                h2A = gA3.tile([128, HFREE], F16, tag="h2A", name="h2A")
                for i, (f0, fw) in enumerate(TCHH):
                    ps = pT.tile([128, 512], F32, tag="tps", name="tps2")
                    nc.tensor.matmul(ps[:, 0:fw], w2[:],
                                     x1A[h][:, f0:f0 + fw],
                                     start=True, stop=True)
                    raw_copy(i + h, h2A[:, f0:f0 + fw], ps[:, 0:fw])
                h2A_l.append(h2A)
            for h in range(2):
                h2B = gB4.tile([128, HFREE], F16, tag="h2B", name="h2B")
                nc.sync.dma_start(h2B[NB:NB + 1, :],
                                  io["b2row"][:, 0:HFREE])
                h2Bv = h2B[0:NB].rearrange("p (gb c) -> p gb c", c=128)
                for q0, q1 in QPC:
                    nc.sync.dma_start(h2Bv[:, q0:q1, :],
                                      h2A_l[h][:, q0 * 128:q1 * 128],
                                      transpose=True)
                h2B_l.append(h2B)
            for h in range(2):
                x2B = gB3.tile([128, HFREE], F16, tag="x2B", name="x2B")
                for i, (f0, fw) in enumerate(TCHH):
                    ps = pM.tile([128, 512], F32, tag="mps", name="mps2")
                    nc.tensor.matmul(ps[:, 0:fw], mixA[0:NB + 1, :],
                                     h2B_l[h][0:NB + 1, f0:f0 + fw],
                                     start=True, stop=True)
                    if (i + h) % 2 == 0:
                        nc.vector.tensor_scalar_max(x2B[:, f0:f0 + fw],
                                                    ps[:, 0:fw], 0.0)
                    else:
                        nc.scalar.activation(x2B[:, f0:f0 + fw],
                                             ps[:, 0:fw], AF.Relu)
                x2B_l.append(x2B)

            # --- L3: mix raw -> B->A -> tf 128->256 (+b3, relu)
            y3B_l, y3A_l, x3A_l = [], [], []
            for h in range(2):
                y3B = gB.tile([128, HFREE], F16, tag="y3B", name="y3B")
                for i, (f0, fw) in enumerate(TCHH):
                    ps = pM.tile([128, 512], F32, tag="mps", name="mps3")
                    nc.tensor.matmul(ps[:, 0:fw], mixZ[:],
                                     x2B_l[h][:, f0:f0 + fw],
                                     start=True, stop=True)
                    raw_copy(i + h, y3B[:, f0:f0 + fw], ps[:, 0:fw])
                y3B_l.append(y3B)
            for h in range(2):
                y3A = gA3.tile([128, HFREE], F16, tag="y3A", name="y3A")
                y3Av = y3A[:].rearrange("c (gb p) -> c gb p", p=128)
                for q0, q1 in QPC:
                    nc.sync.dma_start(y3Av[:, q0:q1, :],
                                      y3B_l[h][:, q0 * 128:q1 * 128],
                                      transpose=True)
                y3A_l.append(y3A)
            # --- L3 tf + L4 per half (L4: tf raw -> T(A->B) x2 ->
            #     acts-stat mix (B->A on PE) -> +b4+relu -> tree sum)
            for h in range(2):
                ho = h * HGB
                x3A = []
                for mt in range(2):
                    xt = gA.tile([128, HFREE], F16, tag="x3A",
                                 name=f"x3A{mt}")
                    for i, (f0, fw) in enumerate(TCHH):
                        ps = pT.tile([128, 512], F32, tag="tps", name="tps3")
                        nc.tensor.matmul(ps[:, 0:fw],
                                         w3[:, mt * 128:(mt + 1) * 128],
                                         y3A_l[h][:, f0:f0 + fw],
                                         start=True, stop=True)
                        relu_copy(i + mt, xt[:, f0:f0 + fw], ps[:, 0:fw],
                                  b3[:, mt:mt + 1])
                    x3A.append(xt)
                x3A_l.append(x3A)
                h4B = []
                for mt in range(2):
                    h4A = gA.tile([128, HFREE], F16, tag="h4A",
                                  name=f"h4A{mt}")
                    for i, (f0, fw) in enumerate(TCHH):
                        ps = pT.tile([128, 512], F32, tag="tps", name="tps4")
                        for kt in range(2):
                            nc.tensor.matmul(
                                ps[:, 0:fw],
                                w4k[kt][:, mt * 128:(mt + 1) * 128],
                                x3A_l[h][kt][:, f0:f0 + fw],
                                start=(kt == 0), stop=(kt == 1))
                        raw_copy(i + mt, h4A[:, f0:f0 + fw], ps[:, 0:fw])
                    hb = gB.tile([128, HFREE], F16, tag="h4B",
                                 name=f"h4B{mt}")
                    hbv = hb[0:NB].rearrange("p (gb c) -> p gb c", c=128)
                    for q0, q1 in QPC:
                        nc.sync.dma_start(hbv[:, q0:q1, :],
                                          h4A[:, q0 * 128:q1 * 128],
                                          transpose=True)
                    h4B.append(hb)
                nsum = g1.tile([128, HGB * 90], F16, tag="nsum",
                               name="nsum")
                for cb in range(2):
                    xt = gSm.tile([128, HGB * NB], F16, tag="x4A",
                                  name=f"x4A{cb}")
                    for g4 in range(HGB // 2):
                        ps = pA.tile([128, 2 * NB], F32, tag="aps",
                                     name="aps")
                        for j in range(2):
                            gb = g4 * 2 + j
                            nc.tensor.matmul(
                                ps[:, j * NB:(j + 1) * NB],
                                h4B[cb][0:NB, gb * 128:gb * 128 + 128],
                                mixA[0:NB, 0:NB], start=True, stop=True)
                        dv = xt[:, g4 * 2 * NB:(g4 + 1) * 2 * NB]
                        relu_copy(g4 + cb, dv, ps[:], b4[:, cb:cb + 1])
                    # tree node-sum into F: 24 nodes -> 12 -> 6 -> 3 -> 1
                    xv = xt[:].rearrange("p (gb blk) -> p gb blk", blk=NB)
                    t1 = nsum[:, 0:HGB * 60].rearrange(
                        "p (gb x) -> p gb x", x=60)
                    t2 = nsum[:, HGB * 60:HGB * 90].rearrange(
                        "p (gb x) -> p gb x", x=30)
                    nc.gpsimd.tensor_tensor(t1, xv[:, :, 0:60],
                                            xv[:, :, 60:120], ALU.add)
                    nc.gpsimd.tensor_tensor(t2, t1[:, :, 0:30],
                                            t1[:, :, 30:60], ALU.add)
                    t3 = t1[:, :, 0:15]
                    nc.gpsimd.tensor_tensor(t3, t2[:, :, 0:15],
                                            t2[:, :, 15:30], ALU.add)
                    dstv = Fts[cb][:, k * TP + ho * G5:
                                   k * TP + (ho + HGB) * G5].rearrange(
                        "p (gb g5) -> p gb g5", g5=G5)
                    nc.gpsimd.tensor_tensor(t2[:, :, 0:5], t3[:, :, 0:5],
                                            t3[:, :, 5:10], ALU.add)
                    nc.gpsimd.tensor_tensor(dstv, t2[:, :, 0:5],
                                            t3[:, :, 10:15], ALU.add)

    # ===== Phase 2: U = F @ (Wih_f/24)^T + (bih+bhh)  (bias folded) =====
    upool = ctx.enter_context(tc.tile_pool(name="upool", bufs=1))
    UCH = [(i * 512, 512) for i in range(4)] + [(2048, 32)]
    Umt = []
    with tc.tile_pool(name="ups", bufs=3, space="PSUM") as ps_u:
        for mt in range(8):
            u = upool.tile([128, FTOT], F16, name=f"U{mt}")
            for f0, fw in UCH:
                ps = ps_u.tile([128, 512], F32, tag="ups", name="ups")
                for kt in range(2):
                    nc.tensor.matmul(ps[:, 0:fw],
                                     lxf[kt][:, mt * 128:(mt + 1) * 128],
                                     Fts[kt][:, f0:f0 + fw],
                                     start=(kt == 0), stop=(kt == 1))
                dst = u[:, f0:f0 + fw]
                src = ps[:, 0:fw]
                if mt % 2:
                    nc.scalar.activation(dst, src, AF.Identity,
                                         bias=bgf[:, mt:mt + 1], scale=1.0)
                else:
                    nc.vector.tensor_scalar_add(dst, src, bgf[:, mt:mt + 1])
            Umt.append(u)

    # ================= Phase 3: forward LSTM (two row groups) ============
    lp = ctx.enter_context(tc.tile_pool(name="lstm", bufs=1))
    NG = 2
    GR = ROWS // NG                      # 452
    Hf = [lp.tile([128, 2 * GR], F16, name=f"Hf{g}") for g in range(NG)]
    Cf = [lp.tile([128, 2 * GR], F16, name=f"Cf{g}") for g in range(NG)]
    gates = [[lp.tile([128, 2 * GR], F16, name=f"g{g}_{p}")
              for p in range(4)] for g in range(NG)]
    tmp = [lp.tile([128, 2 * GR], F16, name=f"tmp{g}") for g in range(NG)]
    tcl = [lp.tile([128, 2 * GR], F16, name=f"tcl{g}") for g in range(NG)]
    for g in range(NG):
        nc.vector.memset(Hf[g][:], 0.0)
        nc.vector.memset(Cf[g][:], 0.0)

    with tc.tile_pool(name="lps", bufs=4, space="PSUM") as ps_l:
        for s in range(WIN):
            k0, par = s // 2, s % 2
            for g in range(NG):
                b0 = g * (BL // NG)
                for mp in range(4):
                    ps = ps_l.tile([128, 1024], F32, tag="lps", name="lps")
                    for j in range(2):
                        mt = 2 * mp + j
                        pslice = ps[:, j * 512:j * 512 + GR]
                        uv = Umt[mt][:].rearrange(
                            "p (b k two) -> p b k two", b=BL, two=2)
                        nc.tensor.matmul(
                            pslice, ident[:],
                            uv[:, b0:b0 + BL // NG, k0:k0 + NW, par],
                            start=True, stop=False)
                        for kt in range(2):
                            nc.tensor.matmul(
                                pslice, lhf[kt][:, mt * 128:(mt + 1) * 128],
                                Hf[g][:, kt * GR:(kt + 1) * GR],
                                start=False, stop=(kt == 1))
                    fn = AF.Sigmoid if mp < 3 else AF.Tanh
                    dstv = gates[g][mp][:].rearrange("p (j r) -> p j r", j=2)
                    psv = ps[:].rearrange("p (j x) -> p j x", j=2)[:, :, 0:GR]
                    nc.scalar.activation(dstv, psv, fn)
                gi, gf, go, tg = gates[g]
                nc.vector.tensor_tensor(tmp[g][:], gi[:], tg[:], ALU.mult)
                nc.vector.tensor_tensor(Cf[g][:], gf[:], Cf[g][:], ALU.mult)
                nc.vector.tensor_tensor(Cf[g][:], Cf[g][:], tmp[g][:],
                                        ALU.add)
                nc.scalar.activation(tcl[g][:], Cf[g][:], AF.Tanh)
                nc.vector.tensor_tensor(Hf[g][:], go[:], tcl[g][:], ALU.mult)

        # ===== Phase 4: backward LSTM single step (only hb[:,0] used) =====
        Hb = [lp.tile([128, 2 * GR], F16, name=f"Hb{g}") for g in range(NG)]
        kb = (WIN - 2) // 2
        for g in range(NG):
            b0 = g * (BL // NG)
            for mp in [0, 2, 3]:          # forget gate irrelevant (c0=0)
                ps = ps_l.tile([128, 1024], F32, tag="lps", name="lpsb")
                for j in range(2):
                    mt = 2 * mp + j
                    pslice = ps[:, j * 512:j * 512 + GR]
                    for kt in range(2):
                        fv = Fts[kt][:].rearrange(
                            "p (b k two) -> p b k two", b=BL, two=2)
                        nc.tensor.matmul(
                            pslice, lxb[kt][:, mt * 128:(mt + 1) * 128],
                            fv[:, b0:b0 + BL // NG, kb:kb + NW, 1],
                            start=(kt == 0), stop=(kt == 1))
                fn = AF.Sigmoid if mp < 3 else AF.Tanh
                for j in range(2):
                    mt = 2 * mp + j
                    nc.scalar.activation(
                        gates[g][mp][:, j * GR:(j + 1) * GR],
                        ps[:, j * 512:j * 512 + GR], fn,
                        bias=bgb[:, mt:mt + 1], scale=1.0)
            gi, gf, go, tg = gates[g]
            nc.vector.tensor_tensor(tmp[g][:], gi[:], tg[:], ALU.mult)
            nc.scalar.activation(tcl[g][:], tmp[g][:], AF.Tanh)
            nc.vector.tensor_tensor(Hb[g][:], go[:], tcl[g][:], ALU.mult)

        # ===== Phase 5: FC head =====
        for g in range(NG):
            ps = ps_l.tile([128, 1024], F32, tag="lps", name=f"lpsf{g}")
            rhs4 = [Hf[g][:, 0:GR], Hf[g][:, GR:2 * GR],
                    Hb[g][:, 0:GR], Hb[g][:, GR:2 * GR]]
            for qt in range(4):
                nc.tensor.matmul(ps[:, 0:GR], wfct[qt][:], rhs4[qt],
                                 start=(qt == 0), stop=(qt == 3))
            ob = lp.tile([EMB, GR], F32, name=f"ob{g}")
            nc.scalar.activation(ob[:], ps[:, 0:GR], AF.Identity,
                                 bias=bfc[:, 0:1], scale=1.0)
            nc.sync.dma_start(io["out_d"][:, g * GR:(g + 1) * GR], ob[:])

    ctx.close()


def _build_program():
    nc = bacc.Bacc("TRN2", target_bir_lowering=False, debug=False,
                   num_devices=NCORES)

    def din(name, shape, dt=F16):
        return nc.dram_tensor(name, shape, dt, kind="ExternalInput").ap()

    io = dict(
        x0B=din("x0B", [NCH, 128, GBLK * C0]),
        mixA=din("mixA", [128, 128]), mixZ=din("mixZ", [128, 128]),
        w1=din("w1", [128, 64]), w2=din("w2", [64, 128]),
        w3=din("w3", [128, 256]), w4=din("w4", [256, 256]),
        b1=din("b1", [64, 1], F32), b2row=din("b2row", [1, AFREE]),
        b3=din("b3", [128, 2], F32), b4=din("b4", [128, 2], F32),
        lxf=din("lxf", [256, 1024]), lhf=din("lhf", [256, 1024]),
        lxb=din("lxb", [256, 1024]),
        bgf=din("bgf", [128, 8], F32), bgb=din("bgb", [128, 8], F32),
        wfc=din("wfc", [512, 128]), bfc=din("bfc", [128, 1], F32),
        ident=din("ident", [128, 128]),
        out_d=nc.dram_tensor("out", [EMB, ROWS], F32,
                             kind="ExternalOutput").ap(),
    )
    with tile.TileContext(nc) as tc:
        _kernel_body(tc, io)
    nc.compile()
    return nc


def _host_prep(inputs):
    f16 = np.float16
    data = np.asarray(inputs["data"], np.float32)
    ei = np.asarray(inputs["edge_index"]).astype(np.int64)

    src = np.concatenate([ei[0], np.arange(N)])
    dst = np.concatenate([ei[1], np.arange(N)])
    deg = np.zeros(N, np.float32)
    np.add.at(deg, dst, 1.0)
    dinv = np.where(deg > 0, deg ** -0.5, 0.0).astype(np.float32)
    Ahat = np.zeros((N, N), np.float32)
    np.add.at(Ahat, (dst, src), dinv[src] * dinv[dst])
    mixZ = np.zeros((128, 128), np.float32)
    mixZ[0:NB, 0:NB] = np.kron(Ahat.T, np.eye(G5, dtype=np.float32))
    mixA = mixZ.copy()
    mixA[NB, 0:NB] = 1.0                 # bias row for L2's mix
    mixA = mixA.astype(f16)
    mixZ = mixZ.astype(f16)

    # x0B: [chunk b][blk = n*5+g5 (120:128 zero)][gb*C0 + c], t = 5*gb+g5
    d = data.reshape(NCORES, BL, T, N, FIN)
    x0B = np.zeros((NCORES, BL, 128, GBLK, C0), np.float32)
    dpad = np.zeros((NCORES, BL, TP, N, FIN), np.float32)
    dpad[:, :, :T] = d
    dv = dpad.reshape(NCORES, BL, GBLK, G5, N, FIN)
    dv = dv.transpose(0, 1, 4, 3, 2, 5).reshape(NCORES, BL, NB, GBLK, FIN)
    x0B[:, :, 0:NB, :, 0:FIN] = dv
    x0B = np.ascontiguousarray(
        x0B.reshape(NCORES, BL, 128, GBLK * C0)).astype(f16)

    perm = np.concatenate([np.arange(0, H), np.arange(H, 2 * H),
                           np.arange(3 * H, 4 * H), np.arange(2 * H, 3 * H)])

    def prep_dir(wih, whh, bih, bhh):
        wihp = np.asarray(wih, np.float32)[perm] / N
        whhp = np.asarray(whh, np.float32)[perm]
        bg = (np.asarray(bih, np.float32) + np.asarray(bhh, np.float32))[perm]
        return (np.ascontiguousarray(wihp.T).astype(f16),
                np.ascontiguousarray(whhp.T).astype(f16),
                np.ascontiguousarray(bg.reshape(8, 128).T).astype(np.float32))

    lxf, lhf, bgf = prep_dir(inputs["lstm_Wih_f"], inputs["lstm_Whh_f"],
                             inputs["lstm_bih_f"], inputs["lstm_bhh_f"])
    lxb, _lhb, bgb = prep_dir(inputs["lstm_Wih_b"], inputs["lstm_Whh_b"],
                              inputs["lstm_bih_b"], inputs["lstm_bhh_b"])

    b2 = np.asarray(inputs["b2"], np.float32)
    b2row = np.tile(b2, GBLK).reshape(1, AFREE).astype(f16)

    com = {
        "mixA": mixA, "mixZ": mixZ,
        "w1": np.tile(np.pad(np.asarray(inputs["W1"], np.float32),
                             ((0, C0 - FIN), (0, 0))), (4, 1)).astype(f16),
        "w2": np.asarray(inputs["W2"], np.float32).astype(f16),
        "w3": np.asarray(inputs["W3"], np.float32).astype(f16),
        "w4": np.asarray(inputs["W4"], np.float32).astype(f16),
        "b1": np.asarray(inputs["b1"], np.float32).reshape(64, 1),
        "b2row": b2row,
        "b3": np.ascontiguousarray(
            np.asarray(inputs["b3"], np.float32).reshape(2, 128).T),
        "b4": np.ascontiguousarray(
            np.asarray(inputs["b4"], np.float32).reshape(2, 128).T),
        "lxf": lxf, "lhf": lhf, "lxb": lxb, "bgf": bgf, "bgb": bgb,
        "wfc": np.asarray(inputs["Wfc"], np.float32).astype(f16),
        "bfc": np.asarray(inputs["bfc"], np.float32).reshape(128, 1),
        "ident": np.eye(128, dtype=f16),
    }
    return [dict(com, x0B=x0B[c]) for c in range(NCORES)]


TRACE = False          # set by test harness to capture an NTFF profile


def kernel(**inputs) -> np.ndarray:
    if "nc" not in _CACHE:
        _CACHE["nc"] = _build_program()
    nc = _CACHE["nc"]
    in_maps = _host_prep(inputs)
    res = bass_utils.run_bass_kernel_spmd(nc, in_maps,
                                          core_ids=list(range(NCORES)),
                                          trace=TRACE)
    _CACHE["last_res"] = res
    outs = []
    for c in range(NCORES):
        o = res.results[c]["out"]                       # [128, 904]
        outs.append(o.reshape(EMB, BL, NW).transpose(1, 2, 0))
    return np.concatenate(outs, 0).astype(np.float32)   # [64, 113, 128]


if __name__ == "__main__":
    import reference
    ins = {k: np.asarray(v) for k, v in reference.setup_inputs().items()}
    out = kernel(**ins)
    print("kernel out", out.shape, out.dtype, float(np.abs(out).max()))
